# revision 12
# baseline (speedup 1.0000x reference)
"""Distributed Bass kernel for nn_Attention_16509854286348.

Strategy (8 NeuronCores, tensor-parallel over heads):
  - Each core owns 2 of the 16 heads: it computes q/k/v projections for
    its 256 output features from the (replicated) input x, applies
    RMSNorm + RoPE (norm weights and the 1/sqrt(dh) attention scale are
    folded into the rope factor tables on the host), runs attention for
    its (batch, head) pairs in bf16, and produces O slices.
  - Attention outputs are transposed on-chip to token-major [tok, 256]
    and redistributed with 4 chunked AllToAlls (1024 tokens each, so
    each core ends up owning 4x128 tokens with ALL 2048 features).
    Each core then computes the full output projection for its tokens
    against a replicated wo^T, producing disjoint 128-token row slices
    that the host concatenates (no host transpose).

Precision: projections in bf16 (fp32 PSUM accumulation); QT/KT stored
bf16 so the scores matmul runs at full bf16 TensorE speed; softmax and
the V path run in bf16 with fp32 statistics.  Scores are O(1) by
construction (RMS-normed q/k), so softmax skips the max subtraction.
The softmax denominator is accumulated on DVE in groups of 4 exp tiles
and reduced with one ones-matmul per group (4 per combo instead of 16),
and 1/den uses the single-op reciprocal_approx_fast.

The attention inner loop is software-pipelined: the scores matmul for
k-block kb+1 is emitted before the PV matmul of block kb, so the PE
never stalls on the ACT exp; the output transposes of each (head, q
chunk) are interleaved into the next combo's matmul stream.
"""

import os
import sys
import types

import numpy as np
import ml_dtypes

import concourse.bass as bass
import concourse.mybir as mybir
import concourse.tile as tile
from concourse.masks import make_identity

# ---------------------------------------------------------------------------
# Environment workarounds
# ---------------------------------------------------------------------------


def _patch_tile_drain():
    """walrus in this image rejects >1 sem wait on the TileContext exit
    drain ("Too many sync wait commands"); split the waits into
    individual single-wait nops on the sync engine."""
    import bass_rust
    from concourse import tile as _tile
    from concourse.vector_clock import ScopedClock

    if getattr(_tile.TileContext, "_drain_patched", False):
        return

    def _drain_and_barrier(self, tick_clock, wait_clock):
        nc = self.nc
        drain_inst = nc.sync.drain()
        wait_clock.add_sem_waits(
            drain_inst.ins, ScopedClock({None: tick_clock.global_clock})
        )
        si = drain_inst.ins.sync_info
        if si is not None and len(si.on_wait) > 1:
            waits = list(si.on_wait)
            updates = list(si.on_update)
            drain_inst.ins.sync_info = bass_rust.SyncInfo(
                on_wait=[], on_update=updates
            )
            for w in waits:
                n = nc.sync.nop(nofuse=True)
                n.ins.sync_info = bass_rust.SyncInfo(on_wait=[w], on_update=[])
        nc.all_engine_barrier()
        assert self.sems is not None
        popped = nc._tile_sem_poison_stack.pop()
        assert popped is self._sem_poison
        nc.clear_and_free_semaphores(list(self.sems.allocated().values()))
        nc.all_engine_barrier()

    _tile.TileContext._drain_and_barrier = _drain_and_barrier
    _tile.TileContext._drain_patched = True


def _legalize_waits(nc, max_waits=1):
    """This image's walrus rejects instructions with more than one sync
    wait ("Too many sync wait commands").  Hoist excess waits onto
    dedicated single-wait nops inserted just before the instruction on
    the same engine — semantically identical, since the engine stalls on
    the nops first."""
    import bass_rust

    counter = [0]
    for f in nc.m.functions:
        for bb in f.blocks:
            changed = False
            newlist = []
            for ins in bb.instructions:
                si = ins.sync_info
                if si is not None and len(si.on_wait) > max_waits:
                    waits = list(si.on_wait)
                    updates = list(si.on_update)
                    for w in waits[:-max_waits]:
                        counter[0] += 1
                        nop = mybir.InstNoOp(
                            name=f"LGW-{counter[0]}", ins=[], outs=[]
                        )
                        nop.engine = ins.engine
                        nop.sync_info = bass_rust.SyncInfo(
                            on_wait=[w], on_update=[]
                        )
                        newlist.append(nop)
                    ins.sync_info = bass_rust.SyncInfo(
                        on_wait=waits[-max_waits:], on_update=updates
                    )
                    changed = True
                newlist.append(ins)
            if changed:
                bb.instructions = newlist


def _register_ntff_hook():
    """The image's antenv package lacks axon_hooks; supply it so
    run_bass_kernel_spmd(trace=True) can profile under axon."""
    if "antenv.axon_hooks" in sys.modules:
        return
    import antenv

    mod = types.ModuleType("antenv.axon_hooks")
    mod._hook = None

    def set_axon_ntff_profile_hook(h):
        mod._hook = h

    def get_axon_ntff_profile_hook():
        return mod._hook

    mod.set_axon_ntff_profile_hook = set_axon_ntff_profile_hook
    mod.get_axon_ntff_profile_hook = get_axon_ntff_profile_hook
    sys.modules["antenv.axon_hooks"] = mod
    antenv.axon_hooks = mod
    try:
        from trn_agent_boot.trn_boot import _ntff_profile_via_ctypes

        mod.set_axon_ntff_profile_hook(
            _ntff_profile_via_ctypes("/opt/axon/libaxon_pjrt.so")
        )
    except Exception:
        pass


# ---------------------------------------------------------------------------
# Problem constants (hardcoded per spec)
# ---------------------------------------------------------------------------

B, S, DM = 2, 2048, 2048
H, DH = 16, 128
EPS = 1e-6
NCORES = 8
HL = H // NCORES            # heads per core = 2
FC = HL * DH                # feature slice per core = 256
TOK = B * S                 # 4096
SB = S // 128               # 16 seq blocks per batch
KB = S // 128               # 16 key blocks per batch
NCH = 4                     # AllToAll chunks (1024 tokens each)
CHT = TOK // NCH            # 1024 tokens per chunk
MYT = CHT // NCORES         # tokens per core per chunk = 128

F32 = mybir.dt.float32
I32 = mybir.dt.int32
BF16 = mybir.dt.bfloat16

LAST_EXEC_NS = None
LAST_RES = None


def _build():
    nc = bass.Bass()
    TT = mybir.AluOpType
    AF = mybir.ActivationFunctionType

    # Host passes weight/rope tensors pre-rearranged partition-major so
    # the loads are single-span contiguous DMAs.
    xt = nc.declare_dram_parameter("xt", [DM, TOK], BF16, isOutput=False)
    wqkv = nc.declare_dram_parameter("wqkv", [128, 16, 3 * FC], BF16,
                                     isOutput=False)
    # full wo^T, partition-major: [128, 16 fblk, 2048 outf]
    wot = nc.declare_dram_parameter("wot", [128, 16, DM], BF16, isOutput=False)
    # rope tables: [128, sb, 2(q/k), 4(F00,F01,F10,F11), 64] fp32
    rope = nc.declare_dram_parameter("rope", [128, SB, 2, 4, 64], BF16,
                                     isOutput=False)
    # output rows owned by this core: chunk ch -> rows [ch*128, (ch+1)*128)
    out_ext = nc.declare_dram_parameter("out", [NCH * MYT, DM], F32,
                                        isOutput=True)

    xt_r = xt.rearrange("(c p) t -> p c t", p=128)        # [128, 16, 4096]

    with tile.TileContext(nc, num_cores=NCORES) as tc:
        from contextlib import ExitStack

        with ExitStack() as ctx:
            const = ctx.enter_context(tc.tile_pool(name="const", bufs=1))
            persist = ctx.enter_context(tc.tile_pool(name="persist", bufs=1))
            xt_pool = ctx.enter_context(tc.tile_pool(name="xtp", bufs=2))
            norm_pool = ctx.enter_context(tc.tile_pool(name="norm", bufs=2))
            exp_pool = ctx.enter_context(tc.tile_pool(name="expp", bufs=3))
            ot_pool = ctx.enter_context(tc.tile_pool(name="otp", bufs=2))
            wo_in = ctx.enter_context(tc.tile_pool(name="woin", bufs=2))
            p_A = ctx.enter_context(
                tc.tile_pool(name="pA", bufs=1, space="PSUM")
            )
            p_tr = ctx.enter_context(tc.tile_pool(name="ptr", bufs=1, space="PSUM"))
            p_o = ctx.enter_context(tc.tile_pool(name="po", bufs=1, space="PSUM"))
            p_den = ctx.enter_context(
                tc.tile_pool(name="pden", bufs=1, space="PSUM")
            )
            dram = ctx.enter_context(tc.tile_pool(name="dram", bufs=1, space="DRAM"))

            # ---- constants (gpsimd queue; sync stays free for stores) ------
            w_sb = [
                const.tile([128, 4, 3 * FC], BF16, name=f"w_sb{g}")
                for g in range(4)
            ]
            nc.gpsimd.dma_start(out=w_sb[0], in_=wqkv[:, 0:4, :])

            def w_ap(ci):
                return w_sb[ci // 4][:, ci % 4]

            # first xt chunk right after the first weight group so the
            # first accumulation can start ASAP
            TOKC = 512
            xt_tiles = {}

            def load_xt(b, tci):
                t = xt_pool.tile([128, 16, TOKC], BF16, tag="xt")
                t0 = b * S + tci * TOKC
                nc.gpsimd.dma_start(out=t, in_=xt_r[:, :, t0 : t0 + TOKC])
                xt_tiles[(b, tci)] = t

            xt_head = const.tile([128, 16, 128], BF16, name="xt_head")
            nc.gpsimd.dma_start(out=xt_head, in_=xt_r[:, :, 0:128])
            load_xt(0, 0)
            for g in range(1, 4):
                nc.gpsimd.dma_start(out=w_sb[g], in_=wqkv[:, 4 * g : 4 * g + 4, :])

            rope_sb = const.tile([128, SB, 2, 4, 64], BF16, name="rope_sb")
            nc.gpsimd.dma_start(out=rope_sb, in_=rope[:])
            # full wo^T (8 MB) is loaded mid-proj(0) (see p0_work) so it
            # does not compete with the startup xt/weight burst
            wot_sb = const.tile([128, 16, DM], BF16, name="wot_sb")
            ones_mat = const.tile([128, 128], BF16, name="ones_mat")
            nc.vector.memset(ones_mat, 1.0)
            ident = const.tile([128, 128], BF16, name="ident")
            make_identity(nc, ident)
            # preload the EXP table on ACT so the first attention combo
            # does not pay the table-load latency
            nc.scalar.activation(out=ident[:, 0:32].bitcast(BF16),
                                 in_=ones_mat[:, 0:32], func=AF.Exp)
            make_identity(nc, ident)
            # warm the PE clock (HAM) while the first DMAs land
            pwu = p_A.tile([128, 512], F32, tag="A", bufs=2)
            for _ in range(48):
                nc.tensor.matmul(pwu[:, 0:128], lhsT=ones_mat, rhs=ones_mat,
                                 start=True, stop=True)

            # ---- persistent tiles (shared across batches) ------------------
            QT = persist.tile([128, HL, S], BF16, name="QT")
            KT = persist.tile([128, HL, S], BF16, name="KT")
            V = persist.tile([128, KB, FC], BF16, name="V")

            a2a_in = [
                dram.tile([CHT, FC], BF16, name=f"a2a_in{ch}")
                for ch in range(NCH)
            ]
            a2a_out = [
                dram.tile([CHT, FC], BF16, name=f"a2a_out{ch}")
                for ch in range(NCH)
            ]

            # =================================================================
            # phase builders
            # =================================================================

            def proj_phase(b, post_tci=None):
                """q/k/v projections + RMSNorm + RoPE + transposes for batch b.
                Writes QT/KT/V (bf16).  Transposes for token block i are
                interleaved into block i+1's matmul stream so the
                transpose-psum never stalls the PE."""
                pending = []  # (j, qr tile, sb) awaiting transpose

                def emit_transpose(j, qr, sb):
                    PT = KT if j >= 2 else QT
                    hl = j % 2
                    ptr = p_tr.tile([128, 128], BF16, tag="tr", bufs=2)
                    nc.tensor.transpose(ptr, qr[:, j], ident)
                    nc.vector.tensor_copy(
                        out=PT[:, hl, sb * 128 : (sb + 1) * 128], in_=ptr
                    )

                for tci in range(S // TOKC):           # 4 chunks of 512 tokens
                    if (b, tci) not in xt_tiles:
                        load_xt(b, tci)
                    xt_sb = xt_tiles.pop((b, tci))
                    # prefetch next chunk
                    nxt = (b, tci + 1) if tci + 1 < S // TOKC else (b + 1, 0)
                    if nxt[0] < B and nxt not in xt_tiles:
                        load_xt(*nxt)
                    for tbl in range(TOKC // 128):
                        sb = tci * (TOKC // 128) + tbl   # seq block 0..15
                        pqA = p_A.tile([128, 512], F32, tag="A", bufs=2)
                        pqV = p_o.tile([128, 512], F32, tag="o", bufs=2)
                        pqB = pqV[:, 0:256]
                        for ci in range(16):
                            if b == 0 and sb == 0:
                                lhsT = xt_head[:, ci]
                            else:
                                lhsT = xt_sb[:, ci, tbl * 128 : (tbl + 1) * 128]
                            nc.tensor.matmul(
                                pqA, lhsT=lhsT, rhs=w_ap(ci)[:, 0:512],
                                start=(ci == 0), stop=(ci == 15),
                            )
                            nc.tensor.matmul(
                                pqB, lhsT=lhsT, rhs=w_ap(ci)[:, 512:768],
                                start=(ci == 0), stop=(ci == 15),
                            )
                            # interleave previous block's transposes between
                            # accumulation steps (every 4th ci); give the
                            # first blocks extra slack while engines warm up
                            if ci % 4 == 3 and pending:
                                if not (b == 0 and sb <= 2 and ci == 3):
                                    emit_transpose(*pending.pop(0))
                        while pending:
                            emit_transpose(*pending.pop(0))

                        # free the psums quickly (ACT; stays on Copy table)
                        qraw = norm_pool.tile([128, 4, 128], F32, tag="qraw")
                        nc.scalar.activation(out=qraw, in_=pqA, func=AF.Copy)
                        nc.scalar.activation(out=V[:, sb, :], in_=pqB,
                                             func=AF.Copy)

                        # rms stats: rstd = rsqrt(mean(t^2)+eps), table-free
                        # Newton on DVE for (q_h0, q_h1, k_h0, k_h1)
                        sqs = norm_pool.tile([128, 4, 128], F32, tag="sqs")
                        ssum = norm_pool.tile([128, 4], F32, tag="ssum")
                        nc.vector.tensor_tensor(
                            out=sqs, in0=qraw, in1=qraw, op=TT.mult
                        )
                        nc.vector.tensor_reduce(
                            out=ssum, in_=sqs, axis=mybir.AxisListType.X,
                            op=TT.add,
                        )
                        v_ = norm_pool.tile([128, 4], F32, tag="v_")
                        nc.vector.tensor_scalar(
                            out=v_, in0=ssum, scalar1=1.0 / DH, scalar2=EPS,
                            op0=TT.mult, op1=TT.add,
                        )
                        y = norm_pool.tile([128, 4], F32, tag="y")
                        t_ = norm_pool.tile([128, 4], F32, tag="t_")
                        u_ = norm_pool.tile([128, 4], F32, tag="u_")
                        # seed: y0 = bits(0x5f3759df - (bits(v) >> 1))
                        nc.vector.tensor_scalar(
                            out=y.bitcast(I32), in0=v_.bitcast(I32),
                            scalar1=1, scalar2=None,
                            op0=TT.logical_shift_right,
                        )
                        nc.vector.tensor_scalar(
                            out=y.bitcast(I32), in0=y.bitcast(I32),
                            scalar1=-1, scalar2=0x5F3759DF,
                            op0=TT.mult, op1=TT.add,
                        )
                        for _ in range(2):  # Newton: y *= 1.5 - 0.5 v y^2
                            nc.vector.tensor_tensor(
                                out=t_, in0=y, in1=y, op=TT.mult
                            )
                            nc.vector.tensor_tensor(
                                out=t_, in0=t_, in1=v_, op=TT.mult
                            )
                            nc.vector.tensor_scalar(
                                out=u_, in0=t_, scalar1=-0.5, scalar2=1.5,
                                op0=TT.mult, op1=TT.add,
                            )
                            nc.vector.tensor_tensor(
                                out=y, in0=y, in1=u_, op=TT.mult
                            )

                        # apply norm (ACT Copy with per-partition scale)
                        qn = norm_pool.tile([128, 4, 128], F32, tag="qn")
                        for j in range(4):
                            nc.scalar.activation(
                                out=qn[:, j], in_=qraw[:, j], func=AF.Copy,
                                scale=y[:, j : j + 1],
                            )
                        # rope: both q/k and both heads per op; bf16 result
                        qr = norm_pool.tile([128, 4, 128], BF16, tag="qr")
                        qn4 = qn.rearrange("p (k h) d -> p k h d", k=2)
                        qr4 = qr.rearrange("p (k h) d -> p k h d", k=2)
                        lo = qn4[:, :, :, 0:64]
                        hi = qn4[:, :, :, 64:128]

                        def f(r):
                            return rope_sb[:, sb, :, None, r, :].to_broadcast(
                                (128, 2, 2, 64)
                            )

                        tmp = norm_pool.tile([128, 2, 2, 64], F32, tag="tmp")
                        nc.vector.tensor_tensor(
                            out=qr4[:, :, :, 0:64], in0=lo, in1=f(0), op=TT.mult
                        )
                        nc.vector.tensor_tensor(
                            out=tmp, in0=hi, in1=f(1), op=TT.mult
                        )
                        nc.vector.tensor_tensor(
                            out=qr4[:, :, :, 0:64], in0=qr4[:, :, :, 0:64],
                            in1=tmp, op=TT.add,
                        )
                        nc.vector.tensor_tensor(
                            out=qr4[:, :, :, 64:128], in0=lo, in1=f(2), op=TT.mult
                        )
                        nc.vector.tensor_tensor(
                            out=tmp, in0=hi, in1=f(3), op=TT.mult
                        )
                        nc.vector.tensor_tensor(
                            out=qr4[:, :, :, 64:128], in0=qr4[:, :, :, 64:128],
                            in1=tmp, op=TT.add,
                        )
                        for j in range(4):
                            pending.append((j, qr, sb))
                    if post_tci is not None:
                        post_tci(tci)
                # flush the final block's transposes
                while pending:
                    emit_transpose(*pending.pop(0))

            # ---- attention -------------------------------------------------

            def make_tr_items(b, qt, hl, ot, dma_eng=None):
                """4 closures: transpose ot [dh, 512q] -> token-major staging
                and DMA into the a2a input buffer."""
                ch = b * 2 + qt // 2
                st = ot_pool.tile([128, 4, 128], BF16, tag="st", bufs=2)
                dst = a2a_in[ch].rearrange("(s p) f -> p s f", p=128)
                s0 = (qt % 2) * 4
                eng = dma_eng if dma_eng is not None else nc.sync

                def item(j):
                    def go():
                        ptr = p_tr.tile([128, 128], BF16, tag="tr", bufs=2)
                        nc.tensor.transpose(
                            ptr, ot[:, j * 128 : (j + 1) * 128], ident
                        )
                        nc.vector.tensor_copy(out=st[:, j], in_=ptr)
                        if j == 3:
                            eng.dma_start(
                                out=dst[:, s0 : s0 + 4,
                                        hl * 128 : (hl + 1) * 128],
                                in_=st,
                            )
                    return go

                return [item(j) for j in range(4)]

            def combo(b, qt, hl, filler):
                """attention for one (batch, head, 512-query chunk); scores
                pipelined one k-block ahead of the PV matmuls; denominator
                via DVE group-accumulation + 1 ones-matmul per 4 blocks.
                `filler` closures are drained into the early k-block slots."""
                filler = list(filler)
                po = p_o.tile([128, 512], F32, tag="o", bufs=2)
                pden = p_den.tile([128, 512], F32, tag="dw", bufs=2)
                ets = {}

                def scores(kb):
                    ps = p_A.tile([128, 512], F32, tag="A", bufs=2)
                    nc.tensor.matmul(
                        ps,
                        lhsT=KT[:, hl, kb * 128 : (kb + 1) * 128],
                        rhs=QT[:, hl, qt * 512 : (qt + 1) * 512],
                        start=True, stop=True,
                    )
                    et = exp_pool.tile([128, 512], BF16, tag="exp")
                    nc.scalar.activation(out=et, in_=ps, func=AF.Exp)
                    ets[kb] = et

                scores(0)
                accs = {}
                pairs = {}
                prev_et = None
                for kb in range(KB):
                    if kb + 1 < KB:
                        scores(kb + 1)
                    et = ets.pop(kb)
                    nc.tensor.matmul(
                        po,
                        lhsT=V[:, kb, hl * 128 : (hl + 1) * 128],
                        rhs=et,
                        start=(kb == 0), stop=(kb == KB - 1),
                    )
                    g, r = divmod(kb, 4)
                    if r % 2 == 0:
                        prev_et = et
                    else:
                        pt = exp_pool.tile([128, 512], BF16, tag="pair",
                                           bufs=2)
                        pairs[r // 2] = pt
                        nc.vector.tensor_tensor(
                            out=pt, in0=prev_et, in1=et, op=TT.add
                        )
                        if r == 3:
                            acc = exp_pool.tile([128, 512], BF16, tag="acc",
                                                bufs=2)
                            accs[g] = acc
                            nc.vector.tensor_tensor(
                                out=acc, in0=pairs[0], in1=pairs[1],
                                op=TT.add,
                            )
                    if r == 0 and g >= 1:  # group g-1 finished one block ago
                        nc.tensor.matmul(
                            pden, lhsT=ones_mat, rhs=accs.pop(g - 1),
                            start=(g == 1), stop=False,
                        )
                    if kb >= 3 and kb % 2 == 1 and filler:
                        filler.pop(0)()
                nc.tensor.matmul(
                    pden, lhsT=ones_mat, rhs=accs.pop(KB // 4 - 1),
                    start=False, stop=True,
                )
                while filler:
                    filler.pop(0)()
                # 1/den via bit-trick seed + 2 Newton iterations (table-free
                # DVE; den is a positive, well-conditioned exp-sum)
                rho = ot_pool.tile([128, 512], F32, tag="recip", bufs=1)
                rt_ = ot_pool.tile([128, 512], F32, tag="rt_", bufs=1)
                nc.vector.tensor_scalar(
                    out=rho.bitcast(I32), in0=pden.bitcast(I32),
                    scalar1=-1, scalar2=0x7EF311C3,
                    op0=TT.mult, op1=TT.add,
                )
                for _ in range(2):  # r = r * (2 - x*r)
                    nc.vector.tensor_tensor(
                        out=rt_, in0=pden, in1=rho, op=TT.mult
                    )
                    nc.vector.tensor_scalar(
                        out=rt_, in0=rt_, scalar1=-1.0, scalar2=2.0,
                        op0=TT.mult, op1=TT.add,
                    )
                    nc.vector.tensor_tensor(
                        out=rho, in0=rho, in1=rt_, op=TT.mult
                    )
                ot = ot_pool.tile([128, 512], BF16, tag="ot")
                nc.vector.tensor_tensor(out=ot, in0=po, in1=rho, op=TT.mult)
                return ot

            def issue_a2a(ch):
                nc.gpsimd.collective_compute(
                    "AllToAll",
                    mybir.AluOpType.bypass,
                    replica_groups=[list(range(NCORES))],
                    ins=[a2a_in[ch].opt()],
                    outs=[a2a_out[ch].opt()],
                )

            def attn_phase(b, post_combo=None, post_flush=None,
                           pre_combo=None):
                """attention for batch b + chunked AllToAll of token-major O.
                post_combo(qt, hl) emits extra work between combos;
                post_flush(ch) runs right after each AllToAll trigger."""
                prev = []
                for qt in range(4):
                    for hl in range(HL):
                        if pre_combo is not None:
                            pre_combo(qt, hl)
                        ot = combo(b, qt, hl, prev)
                        if qt % 2 == 1 and hl == HL - 1:
                            # flush own transposes on the gpsimd queue (sync
                            # may be backlogged with osb stores), then
                            # redistribute
                            prev = make_tr_items(b, qt, hl, ot,
                                                 dma_eng=nc.gpsimd)
                            while prev:
                                prev.pop(0)()
                            issue_a2a(b * 2 + qt // 2)
                            if post_flush is not None:
                                post_flush(b * 2 + qt // 2)
                        else:
                            prev = make_tr_items(b, qt, hl, ot)
                        if post_combo is not None:
                            post_combo(qt, hl)

            # ---- output projection (token-sharded after AllToAll) ----------

            OT_tiles = {}

            def wo_recv_dma(ch):
                rt = wo_in.tile([128, 8, 2, 128], BF16, tag="rt", bufs=1)
                src = a2a_out[ch].rearrange("(s t) (h f) -> t s h f",
                                            t=128, h=2)
                nc.gpsimd.dma_start(out=rt, in_=src)
                OT_tiles[ch] = ("rt", rt)

            def wo_recv_tr(ch):
                kind, rt = OT_tiles[ch]
                assert kind == "rt"
                OT_t = wo_in.tile([128, 16, 128], BF16, tag="OT", bufs=1)
                for g in range(16):
                    ptr = p_tr.tile([128, 128], BF16, tag="tr", bufs=2)
                    nc.tensor.transpose(ptr, rt[:, g // 2, g % 2], ident)
                    nc.vector.tensor_copy(out=OT_t[:, g], in_=ptr)
                OT_tiles[ch] = ("ot", OT_t)

            def wo_of(ch, of):
                kind, OT_t = OT_tiles[ch]
                assert kind == "ot"
                pw = p_den.tile([128, 512], F32, tag="dw", bufs=2)
                for g in range(16):
                    nc.tensor.matmul(
                        pw,
                        lhsT=OT_t[:, g],
                        rhs=wot_sb[:, g, of * 512 : (of + 1) * 512],
                        start=(g == 0), stop=(g == 15),
                    )
                osb = ot_pool.tile([128, 512], F32, tag="osb", bufs=2)
                nc.vector.tensor_copy(out=osb, in_=pw)
                nc.sync.dma_start(
                    out=out_ext[ch * 128 : (ch + 1) * 128,
                                of * 512 : (of + 1) * 512],
                    in_=osb,
                )

            # ---- emission order (controls per-engine instruction order) ----
            def p0_work(tci):
                if tci == 1:
                    nc.gpsimd.dma_start(out=wot_sb, in_=wot[:])

            proj_phase(0, post_tci=p0_work)
            attn_phase(0)                    # issues A2A 0, 1

            def p1_work(tci):
                if tci == 0:
                    wo_recv_dma(0)
                elif tci == 1:
                    wo_recv_tr(0)
                    wo_of(0, 0)
                    wo_of(0, 1)
                elif tci == 2:
                    wo_of(0, 2)
                    wo_of(0, 3)
                    wo_recv_dma(1)
                else:
                    wo_recv_tr(1)
                    wo_of(1, 0)
                    wo_of(1, 1)

            proj_phase(1, post_tci=p1_work)

            def a1_work(qt, hl):
                if qt == 0 and hl == 0:
                    wo_of(1, 2)
                elif qt == 0 and hl == 1:
                    wo_of(1, 3)
                elif qt == 2 and hl == 0:
                    wo_recv_dma(2)
                elif qt == 2 and hl == 1:
                    wo_recv_tr(2)
                    wo_of(2, 0)
                elif qt == 3 and hl == 0:
                    wo_of(2, 1)
                elif qt == 3 and hl == 1:    # A2A(3) just issued
                    wo_of(2, 2)
                    wo_of(2, 3)

            attn_phase(1, post_combo=a1_work)
            wo_recv_dma(3)
            wo_recv_tr(3)
            for of in range(4):
                wo_of(3, of)

    return nc


def _prep_inputs(x, rope_emb, wq, wk, wv, wo, q_norm_w, k_norm_w):
    """Host-side shard prep: per-core input maps."""
    bf = ml_dtypes.bfloat16
    X = np.ascontiguousarray(x.reshape(TOK, DM))
    xt = np.ascontiguousarray(X.T).astype(bf)  # [DM, TOK]

    gamma = 1.0 / np.sqrt(DH)
    qw = np.asarray(q_norm_w, np.float32)
    kw = np.asarray(k_norm_w, np.float32)
    fr = np.asarray(rope_emb, np.float32)[:, 0]  # [S, 64, 2, 2]

    def rope_pack(w, scale):
        # F[r] for r=(i,l): out[i*64+j] += F[i,l][s,j] * t[l*64+j], t = norm*w
        F = np.empty((S, 4, 64), np.float32)
        F[:, 0] = fr[:, :, 0, 0] * w[None, :64] * scale
        F[:, 1] = fr[:, :, 0, 1] * w[None, 64:] * scale
        F[:, 2] = fr[:, :, 1, 0] * w[None, :64] * scale
        F[:, 3] = fr[:, :, 1, 1] * w[None, 64:] * scale
        return F

    rope_all = np.stack([rope_pack(qw, gamma), rope_pack(kw, 1.0)], axis=1)
    # [S, 2, 4, 64] -> partition-major [128, SB, 2, 4, 64]
    rope_pm = np.ascontiguousarray(
        rope_all.reshape(SB, 128, 2, 4, 64).transpose(1, 0, 2, 3, 4)
    ).astype(bf)

    def pmajor(a):
        # [DM, F] -> [128, 16, F] with dm = c*128 + p
        return np.ascontiguousarray(
            a.reshape(16, 128, a.shape[1]).transpose(1, 0, 2)
        )

    # full wo^T [feature, out_feature], partition-major over features
    wot_pm = pmajor(np.ascontiguousarray(np.asarray(wo, np.float32).T)
                    .astype(bf))

    in_maps = []
    for c in range(NCORES):
        rows = slice(c * FC, (c + 1) * FC)
        wqkv = np.concatenate(
            [wq[rows].T, wk[rows].T, wv[rows].T], axis=1
        ).astype(bf)  # [DM, 768]
        in_maps.append(
            {
                "xt": xt,
                "wqkv": pmajor(wqkv),
                "wot": wot_pm,
                "rope": rope_pm,
            }
        )
    return in_maps


_CACHE = {}


def kernel(x, rope_emb, wq, wk, wv, wo, q_norm_w, k_norm_w):
    global LAST_EXEC_NS, LAST_RES
    x = np.asarray(x, np.float32)
    rope_emb = np.asarray(rope_emb, np.float32)
    wq = np.asarray(wq, np.float32)
    wk = np.asarray(wk, np.float32)
    wv = np.asarray(wv, np.float32)
    wo = np.asarray(wo, np.float32)
    q_norm_w = np.asarray(q_norm_w, np.float32)
    k_norm_w = np.asarray(k_norm_w, np.float32)
    _patch_tile_drain()
    _register_ntff_hook()
    from concourse.bass_utils import run_bass_kernel_spmd

    if "nc" not in _CACHE:
        nc = _build()
        _legalize_waits(nc)
        _CACHE["nc"] = nc
    nc = _CACHE["nc"]

    in_maps = _prep_inputs(x, rope_emb, wq, wk, wv, wo, q_norm_w, k_norm_w)
    trace = os.environ.get("ATTN_TRACE", "0") == "1"
    res = run_bass_kernel_spmd(
        nc, in_maps, core_ids=list(range(NCORES)), trace=trace
    )
    LAST_EXEC_NS = res.exec_time_ns
    LAST_RES = res

    # core c's out rows: chunk ch covers tokens [ch*1024 + c*128, +128)
    out = np.empty((TOK, DM), np.float32)
    for c in range(NCORES):
        oc = res.results[c]["out"]
        for ch in range(NCH):
            t0 = ch * CHT + c * MYT
            out[t0 : t0 + MYT] = oc[ch * MYT : (ch + 1) * MYT]
    return np.ascontiguousarray(out.reshape(B, S, DM), dtype=np.float32)
